# revision 7
# baseline (speedup 1.0000x reference)
"""Causal self-attention (QKV GEMM + RoPE + causal softmax attention + output
projection) for Trainium2, sharded over 8 NeuronCores.

Sharding: tensor-parallel over heads (2 heads/core). Each core computes the
QKV projections for its heads (full token range), RoPE, causal attention, and
a partial output projection over its heads' channels; the host sums the 8
partial projections (the only cross-core reduction) and reshapes.

v3: single 8-bank PSUM layout alive for the whole program; attention j-chunks
are emitted interleaved into the QKV phase (sub-tile deps let scores start as
soon as the needed qkT/v ranges land); score matmuls run one tile ahead of
the AV/sum matmuls across all (head, chunk) boundaries; diagonal tiles use
exact-causal shrunken extents with a single [128,128] triangular mask; the
softmax denominator uses an all-ones [128,128] stationary; DRAM I/O is fp16.
"""

import os
import sys
from collections import deque

import numpy as np


def _ensure_concourse():
    try:
        import concourse.bass  # noqa: F401
        return
    except ImportError:
        pass
    for p in (
        "/opt/trn_rl_repo",
        os.path.expanduser("~/.axon_site/_ro/trn_rl_repo"),
        "/root/.axon_site/_ro/trn_rl_repo",
    ):
        if os.path.isdir(p) and p not in sys.path:
            sys.path.insert(0, p)
    import concourse.bass  # noqa: F401


# Problem shape (hardcoded per contract)
B, T, C, H = 2, 2048, 2048, 16
D, RD = 128, 64
NCORES = 8
HPC = H // NCORES          # heads per core = 2
BT = B * T                 # 4096
P = 128
MT = T // P                # 16 token tiles per batch
KTC = C // P               # 16 contraction tiles over C
KH = KTC // 2              # kt per xc half-load
FPC = 3 * HPC * D          # 768 qkv features per core
NQ = 512                   # query chunk
NJ = T // NQ               # 4 query chunks per instance
SCALE = 1.0 / float(np.sqrt(D))

_PROGRAM = None


def _build_program():
    _ensure_concourse()
    from contextlib import ExitStack

    import concourse.bacc as bacc
    import concourse.mybir as mybir
    import concourse.tile as tile
    from concourse.alu_op_type import AluOpType
    from concourse.masks import make_identity

    F32 = mybir.dt.float32
    F16 = mybir.dt.float16
    EXP = mybir.ActivationFunctionType.Exp
    MUL = AluOpType.mult
    SUB = AluOpType.subtract
    ADD = AluOpType.add
    PSUM = "PSUM"

    nc = bacc.Bacc("TRN2", target_bir_lowering=False, debug=False,
                   num_devices=NCORES)

    xt_d = nc.dram_tensor("xt", [C, BT], F16, kind="ExternalInput").ap()
    w_d = nc.dram_tensor("wqkv", [P, KTC * FPC], F16, kind="ExternalInput").ap()
    cos_d = nc.dram_tensor("cosw", [P, (BT // P) * RD], F16, kind="ExternalInput").ap()
    sin_d = nc.dram_tensor("sinw", [P, (BT // P) * RD], F16, kind="ExternalInput").ap()
    tri_d = nc.dram_tensor("trid", [P, P], F16, kind="ExternalInput").ap()
    wp_d = nc.dram_tensor("wproj", [P, HPC * C], F16, kind="ExternalInput").ap()
    out_d = nc.dram_tensor("outp", [BT, C], F16, kind="ExternalOutput").ap()

    xt_r = xt_d.rearrange("(kt p) t -> p kt t", p=P)
    GM = 2                    # m-tiles per phase-A group
    NG = MT // GM             # 8 groups per batch
    WCH = 4                   # kt per w DMA chunk

    with tile.TileContext(nc) as tc, ExitStack() as gctx:
        ep = gctx.enter_context

        const = ep(tc.tile_pool(name="const", bufs=1))
        tri_sb = const.tile([P, P], F16, tag="tri")
        cos_sb = const.tile([P, (BT // P) * RD], F16, tag="cos")
        sin_sb = const.tile([P, (BT // P) * RD], F16, tag="sin")
        ident = const.tile([P, P], F32, tag="ident")
        ones_cf = const.tile([P, P], F32, tag="ones_cf")
        ones_c = const.tile([P, P], F16, tag="ones_c")

        nc.gpsimd.dma_start(out=tri_sb[:], in_=tri_d)
        nc.gpsimd.dma_start(out=cos_sb[:], in_=cos_d)
        nc.gpsimd.dma_start(out=sin_sb[:], in_=sin_d)
        make_identity(nc, ident[:])
        nc.vector.memset(ones_cf[:], 1.0)
        nc.vector.tensor_copy(ones_c[:], ones_cf[:])

        # weights, split-loaded per chunk so the first matmuls start early
        wpool = ep(tc.tile_pool(name="wqkv", bufs=1))
        w_sb = wpool.tile([P, KTC * FPC], F16, tag="w")
        for wc in range(KTC // WCH):
            nc.sync.dma_start(
                out=w_sb[:, wc * WCH * FPC:(wc + 1) * WCH * FPC],
                in_=w_d[:, wc * WCH * FPC:(wc + 1) * WCH * FPC])
        wppool = ep(tc.tile_pool(name="wp", bufs=1))
        wp_sb = wppool.tile([P, HPC * C], F16, tag="wp")
        nc.gpsimd.dma_start(out=wp_sb[:], in_=wp_d)

        # persistent SBUF pools
        qkt_pool = ep(tc.tile_pool(name="qkt", bufs=2))
        v_pool = ep(tc.tile_pool(name="v", bufs=2))
        yt_pool = ep(tc.tile_pool(name="yt", bufs=2))
        xcol = ep(tc.tile_pool(name="xcol", bufs=4))
        rotp = ep(tc.tile_pool(name="rot", bufs=2))
        tmpp = ep(tc.tile_pool(name="tmp", bufs=2))
        attnp = ep(tc.tile_pool(name="attn", bufs=4))
        rcpp = ep(tc.tile_pool(name="rcp", bufs=2))
        outrow = ep(tc.tile_pool(name="orow", bufs=2))

        # PSUM pools: 2 + 2 + 1 + 2 + 1 = 8 banks, alive for the whole program
        pa = ep(tc.tile_pool(name="pa", bufs=2, space=PSUM))    # A q|k + C proj
        ps = ep(tc.tile_pool(name="ps", bufs=2, space=PSUM))    # B sc + A v + C proj
        pt = ep(tc.tile_pool(name="pt", bufs=1, space=PSUM))    # A transposes
        py = ep(tc.tile_pool(name="py", bufs=2, space=PSUM))    # B y accum
        pm = ep(tc.tile_pool(name="pm", bufs=1, space=PSUM))    # B softmax sums

        for b in range(B):
            qkT = qkt_pool.tile([P, 4 * T], F16, tag="qkT", name=f"qkT_{b}")
            v_sb = v_pool.tile([P, MT * HPC * D], F16, tag="v", name=f"v_{b}")
            yt = yt_pool.tile([P, HPC * T], F16, tag="yt", name=f"yt_{b}")
            qv = qkT[:].rearrange("p (hb t) -> p hb t", hb=4)

            # -------- emission helpers --------
            def emit_A_group(g):
                halves = []
                for hf in range(2):
                    xc = xcol.tile([P, KH, GM * P], F16, tag="xc",
                                   name=f"xc_{b}_{g}_{hf}")
                    nc.sync.dma_start(
                        out=xc[:],
                        in_=xt_r[:, hf * KH:(hf + 1) * KH,
                                 b * T + g * GM * P: b * T + (g + 1) * GM * P])
                    halves.append(xc)
                p5s = []
                pv = ps.tile([P, NQ], F32, tag="sc", name=f"pv_{b}_{g}")
                for mi in range(GM):
                    p5 = pa.tile([P, NQ], F32, tag="qk", name=f"p5_{b}_{g}_{mi}")
                    p5s.append(p5)
                    for kt in range(KTC):
                        lhsT = halves[kt // KH][:, kt % KH, mi * P:(mi + 1) * P]
                        nc.tensor.matmul(
                            p5[:], lhsT,
                            w_sb[:, kt * FPC: kt * FPC + 512],
                            start=(kt == 0), stop=(kt == KTC - 1))
                        nc.tensor.matmul(
                            pv[:, mi * 256:(mi + 1) * 256], lhsT,
                            w_sb[:, kt * FPC + 512:(kt + 1) * FPC],
                            start=(kt == 0 and mi == 0), stop=(kt == KTC - 1),
                            skip_group_check=True)
                # RoPE + transposes per m-tile of the group
                for mi in range(GM):
                    m = g * GM + mi
                    gm = b * MT + m
                    p5 = p5s[mi]
                    rot = rotp.tile([P, NQ], F32, tag="rot",
                                    name=f"rot_{b}_{g}_{mi}")
                    p3 = p5[:].rearrange("p (blk two d) -> p blk two d",
                                         two=2, d=RD)
                    re_, im_ = p3[:, :, 0, :], p3[:, :, 1, :]
                    r3 = rot[:].rearrange("p (blk two d) -> p blk two d",
                                          two=2, d=RD)
                    cosb = (cos_sb[:, gm * RD:(gm + 1) * RD]
                            .unsqueeze(1).broadcast_to([P, 4, RD]))
                    sinb = (sin_sb[:, gm * RD:(gm + 1) * RD]
                            .unsqueeze(1).broadcast_to([P, 4, RD]))
                    t1 = tmpp.tile([P, 256], F32, tag="t1")
                    t2 = tmpp.tile([P, 256], F32, tag="t2")
                    t1v = t1[:].rearrange("p (blk d) -> p blk d", d=RD)
                    t2v = t2[:].rearrange("p (blk d) -> p blk d", d=RD)
                    nc.vector.tensor_tensor(t1v, re_, cosb, MUL)
                    nc.vector.tensor_tensor(t2v, im_, sinb, MUL)
                    nc.vector.tensor_tensor(r3[:, :, 0, :], t1v, t2v, SUB)
                    t3 = tmpp.tile([P, 256], F32, tag="t3")
                    t4 = tmpp.tile([P, 256], F32, tag="t4")
                    t3v = t3[:].rearrange("p (blk d) -> p blk d", d=RD)
                    t4v = t4[:].rearrange("p (blk d) -> p blk d", d=RD)
                    nc.vector.tensor_tensor(t3v, re_, sinb, MUL)
                    nc.vector.tensor_tensor(t4v, im_, cosb, MUL)
                    nc.vector.tensor_tensor(r3[:, :, 1, :], t3v, t4v, ADD)
                    tp = pt.tile([P, 4 * P], F32, tag="tp",
                                 name=f"tp_{b}_{g}_{mi}")
                    for hb in range(4):
                        nc.tensor.matmul(
                            tp[:, hb * P:(hb + 1) * P],
                            rot[:, hb * P:(hb + 1) * P], ident[:],
                            is_transpose=True,
                            start=(hb == 0), stop=(hb == 3),
                            skip_group_check=True)
                    tpv = tp[:].rearrange("p (hb t) -> p hb t", hb=4)
                    nc.scalar.copy(qv[:, :, m * P:(m + 1) * P], tpv)
                nc.scalar.copy(
                    v_sb[:, g * GM * HPC * D:(g + 1) * GM * HPC * D], pv[:])

            # -------- phase B/C with a cross-boundary software pipeline -----
            bq = deque()
            state = {"ncopy": 0}

            def emit_C_chunk(j):
                for mi in range(4):
                    m = 4 * j + mi
                    orow = outrow.tile([P, C], F16, tag="orow",
                                       name=f"orow_{b}_{m}")
                    for oc in range(4):
                        pool = pa if oc % 2 == 0 else ps
                        tag = "qk" if oc % 2 == 0 else "sc"
                        op = pool.tile([P, NQ], F32, tag=tag,
                                       name=f"op_{b}_{m}_{oc}")
                        for h in range(HPC):
                            nc.tensor.matmul(
                                op[:],
                                yt[:, h * T + m * P: h * T + (m + 1) * P],
                                wp_sb[:, h * C + oc * 512:
                                      h * C + (oc + 1) * 512],
                                start=(h == 0), stop=(h == HPC - 1))
                        if state["ncopy"] % 2 == 0:
                            nc.scalar.copy(orow[:, oc * 512:(oc + 1) * 512],
                                           op[:])
                        else:
                            nc.vector.tensor_copy(
                                orow[:, oc * 512:(oc + 1) * 512], op[:])
                        state["ncopy"] += 1
                    nc.gpsimd.dma_start(
                        out=out_d[(b * MT + m) * P:(b * MT + m + 1) * P, :],
                        in_=orow[:])

            def pump():
                (h, j, kt, nkt, at, y_ps, s_ps, fr, qo) = bq.popleft()
                nc.tensor.matmul(
                    y_ps[:, qo:NQ],
                    v_sb[:, kt * HPC * D + h * D: kt * HPC * D + (h + 1) * D],
                    at[:, 0:fr], start=(kt == 0), stop=(kt == nkt - 1),
                    skip_group_check=True)
                nc.tensor.matmul(
                    s_ps[:, qo:NQ], ones_c[:], at[:, 0:fr],
                    start=(kt == 0), stop=(kt == nkt - 1),
                    skip_group_check=True)
                if kt == nkt - 1:
                    inst = b * HPC + h
                    rcp = rcpp.tile([P, NQ], F32, tag="rc",
                                    name=f"rc_{inst}_{j}")
                    with nc.allow_low_precision(reason="softmax recip"):
                        nc.vector.reciprocal_approx_fast(out=rcp[:],
                                                         in_=s_ps[:])
                    nc.vector.tensor_tensor(
                        yt[:, h * T + j * NQ: h * T + (j + 1) * NQ],
                        y_ps[:], rcp[:], MUL)
                    if h == HPC - 1:
                        emit_C_chunk(j)

            def emit_B_chunk(j):
                nkt = 4 * (j + 1)
                for h in range(HPC):
                    inst = b * HPC + h
                    y_ps = py.tile([P, NQ], F32, tag="y", name=f"y_{inst}_{j}")
                    s_ps = pm.tile([P, NQ], F32, tag="s", name=f"s_{inst}_{j}")
                    for kt in range(nkt):
                        kd = kt - 4 * j
                        fr = NQ if kd < 0 else NQ - kd * P
                        qo = 0 if kd < 0 else kd * P
                        sc = ps.tile([P, NQ], F32, tag="sc",
                                     name=f"sc_{inst}_{j}_{kt}")
                        nc.tensor.matmul(
                            sc[:, 0:fr],
                            qkT[:, (2 + h) * T + kt * P:
                                (2 + h) * T + (kt + 1) * P],
                            qkT[:, h * T + j * NQ + qo: h * T + (j + 1) * NQ],
                            start=True, stop=True)
                        at = attnp.tile([P, NQ], F16, tag="at",
                                        name=f"at_{inst}_{j}_{kt}")
                        nc.scalar.activation(at[:, 0:fr], sc[:, 0:fr], EXP,
                                             scale=SCALE)
                        if kd >= 0:
                            nc.gpsimd.tensor_tensor(
                                at[:, 0:P], at[:, 0:P], tri_sb[:], MUL)
                        bq.append((h, j, kt, nkt, at, y_ps, s_ps, fr, qo))
                        while len(bq) >= 2:
                            pump()

            # -------- batch emission schedule --------
            for g in range(NG):
                emit_A_group(g)
                if g % 2 == 1:
                    emit_B_chunk(g // 2)
            while bq:
                pump()

    nc.compile()
    return nc


def _perm(rows):
    return np.concatenate([rows[0::2], rows[1::2]], axis=0)


def _host_inputs(x, mask, freqs_cos, freqs_sin, w_attn, w_proj):
    f16 = np.float16
    f32 = np.float32
    x = np.asarray(x, f32)
    fc = np.asarray(freqs_cos, f32)
    fs = np.asarray(freqs_sin, f32)
    w_attn = np.asarray(w_attn, f32)
    w_proj = np.asarray(w_proj, f32)

    xT = np.ascontiguousarray(x.reshape(BT, C).T.astype(f16))

    def rows_arrange(a):  # [BT, RD] -> [P, (BT//P)*RD]
        return np.ascontiguousarray(
            a.reshape(BT // P, P, RD).transpose(1, 0, 2).reshape(P, -1)
            .astype(f16))

    cosw = rows_arrange(np.concatenate([fc] * B, axis=0))
    sinw = rows_arrange(np.concatenate([fs] * B, axis=0))

    # triangular keep-mask in [k, q] orientation: keep iff k' <= q'
    trid = np.triu(np.ones((P, P), dtype=f16))
    trid = np.ascontiguousarray(trid)

    wq, wk, wv = w_attn[0:C], w_attn[C:2 * C], w_attn[2 * C:3 * C]
    in_maps = []
    for c in range(NCORES):
        h0, h1 = HPC * c, HPC * c + 1
        Wc = np.concatenate([
            _perm(wq[h0 * D:(h0 + 1) * D]), _perm(wq[h1 * D:(h1 + 1) * D]),
            _perm(wk[h0 * D:(h0 + 1) * D]), _perm(wk[h1 * D:(h1 + 1) * D]),
            wv[h0 * D:(h0 + 1) * D], wv[h1 * D:(h1 + 1) * D]], axis=0)
        wqkv_c = np.ascontiguousarray(
            Wc.T.reshape(KTC, P, FPC).transpose(1, 0, 2).reshape(P, KTC * FPC)
            .astype(f16))
        wp_c = w_proj[:, c * HPC * D:(c + 1) * HPC * D].T  # [256, C]
        wp_c = np.ascontiguousarray(
            wp_c.reshape(HPC, P, C).transpose(1, 0, 2).reshape(P, HPC * C)
            .astype(f16))
        in_maps.append({
            "xt": xT, "wqkv": wqkv_c, "cosw": cosw, "sinw": sinw,
            "trid": trid, "wproj": wp_c,
        })
    return in_maps


def kernel(x, mask, freqs_cos, freqs_sin, w_attn, w_proj):
    global _PROGRAM
    _ensure_concourse()
    from concourse.bass_utils import run_bass_kernel_spmd

    if _PROGRAM is None:
        _PROGRAM = _build_program()
    nc = _PROGRAM

    in_maps = _host_inputs(x, mask, freqs_cos, freqs_sin, w_attn, w_proj)
    res = run_bass_kernel_spmd(nc, in_maps, list(range(NCORES)))
    out = res.results[0]["outp"].astype(np.float32)
    for i in range(1, NCORES):
        out = out + res.results[i]["outp"].astype(np.float32)
    return np.ascontiguousarray(out.reshape(B, T, C))


# revision 12
# speedup vs baseline: 1.0098x; 1.0098x over previous
"""Causal self-attention (QKV GEMM + RoPE + causal softmax attention + output
projection) for Trainium2, sharded over 8 NeuronCores.

Sharding: tensor-parallel over heads (2 heads/core). Each core computes the
QKV projections for its heads (full token range), RoPE, causal attention, and
a partial output projection over its heads' channels; the host sums the 8
partial projections (the only cross-core reduction) and reshapes.

v3: single 8-bank PSUM layout alive for the whole program; attention j-chunks
are emitted interleaved into the QKV phase (sub-tile deps let scores start as
soon as the needed qkT/v ranges land); score matmuls run one tile ahead of
the AV/sum matmuls across all (head, chunk) boundaries; diagonal tiles use
exact-causal shrunken extents with a single [128,128] triangular mask; the
softmax denominator uses an all-ones [128,128] stationary; DRAM I/O is fp16.
"""

import os
import sys
from collections import deque

import numpy as np


def _ensure_concourse():
    try:
        import concourse.bass  # noqa: F401
        return
    except ImportError:
        pass
    for p in (
        "/opt/trn_rl_repo",
        os.path.expanduser("~/.axon_site/_ro/trn_rl_repo"),
        "/root/.axon_site/_ro/trn_rl_repo",
    ):
        if os.path.isdir(p) and p not in sys.path:
            sys.path.insert(0, p)
    import concourse.bass  # noqa: F401


# Problem shape (hardcoded per contract)
B, T, C, H = 2, 2048, 2048, 16
D, RD = 128, 64
NCORES = 8
HPC = H // NCORES          # heads per core = 2
BT = B * T                 # 4096
P = 128
MT = T // P                # 16 token tiles per batch
KTC = C // P               # 16 contraction tiles over C
KH = KTC // 2              # kt per xc half-load
FPC = 3 * HPC * D          # 768 qkv features per core
NQ = 512                   # query chunk
NJ = T // NQ               # 4 query chunks per instance
SCALE = 1.0 / float(np.sqrt(D))

_PROGRAM = None


def _build_program():
    _ensure_concourse()
    from contextlib import ExitStack

    import concourse.bacc as bacc
    import concourse.mybir as mybir
    import concourse.tile as tile
    from concourse.alu_op_type import AluOpType
    from concourse.masks import make_identity

    F32 = mybir.dt.float32
    F16 = mybir.dt.float16
    EXP = mybir.ActivationFunctionType.Exp
    MUL = AluOpType.mult
    SUB = AluOpType.subtract
    ADD = AluOpType.add
    PSUM = "PSUM"

    nc = bacc.Bacc("TRN2", target_bir_lowering=False, debug=False,
                   num_devices=NCORES)

    xt_d = nc.dram_tensor("xt", [C, BT], F16, kind="ExternalInput").ap()
    w_d = nc.dram_tensor("wqkv", [P, KTC * FPC], F16, kind="ExternalInput").ap()
    cos_d = nc.dram_tensor("cosw", [P, (BT // P) * RD], F16, kind="ExternalInput").ap()
    sin_d = nc.dram_tensor("sinw", [P, (BT // P) * RD], F16, kind="ExternalInput").ap()
    tri_d = nc.dram_tensor("trid", [P, P], F16, kind="ExternalInput").ap()
    wp_d = nc.dram_tensor("wproj", [P, HPC * C], F16, kind="ExternalInput").ap()
    out_d = nc.dram_tensor("outp", [BT, C], F16, kind="ExternalOutput").ap()

    xt_r = xt_d.rearrange("(kt p) t -> p kt t", p=P)
    GM = 2                    # m-tiles per phase-A group
    NG = MT // GM             # 8 groups per batch
    WCH = 4                   # kt per w DMA chunk

    with tile.TileContext(nc) as tc, ExitStack() as gctx:
        ep = gctx.enter_context

        const = ep(tc.tile_pool(name="const", bufs=1))
        tri_sb = const.tile([P, P], F16, tag="tri")
        cos_sb = const.tile([P, (BT // P) * RD], F16, tag="cos")
        sin_sb = const.tile([P, (BT // P) * RD], F16, tag="sin")
        ident = const.tile([P, P], F16, tag="ident")
        ones_cf = const.tile([P, P], F32, tag="ones_cf")
        ones_c = const.tile([P, P], F16, tag="ones_c")

        # weights first on the gpsimd queue: the first matmuls need them
        wpool = ep(tc.tile_pool(name="wqkv", bufs=1))
        w_sb = wpool.tile([P, KTC * FPC], F16, tag="w")
        for wc in range(KTC // WCH):
            nc.gpsimd.dma_start(
                out=w_sb[:, wc * WCH * FPC:(wc + 1) * WCH * FPC],
                in_=w_d[:, wc * WCH * FPC:(wc + 1) * WCH * FPC])
        nc.gpsimd.dma_start(out=tri_sb[:], in_=tri_d)
        nc.gpsimd.dma_start(out=cos_sb[:], in_=cos_d)
        nc.gpsimd.dma_start(out=sin_sb[:], in_=sin_d)
        make_identity(nc, ident[:])
        nc.vector.memset(ones_cf[:], 1.0)
        nc.vector.tensor_copy(ones_c[:], ones_cf[:])
        wppool = ep(tc.tile_pool(name="wp", bufs=1))
        wp_sb = wppool.tile([P, HPC * C], F16, tag="wp")
        nc.gpsimd.dma_start(out=wp_sb[:], in_=wp_d)

        # persistent SBUF pools
        qkt_pool = ep(tc.tile_pool(name="qkt", bufs=2))
        v_pool = ep(tc.tile_pool(name="v", bufs=2))
        yt_pool = ep(tc.tile_pool(name="yt", bufs=2))
        xcol = ep(tc.tile_pool(name="xcol", bufs=4))
        rotp = ep(tc.tile_pool(name="rot", bufs=2))
        tmpp = ep(tc.tile_pool(name="tmp", bufs=2))
        attnp = ep(tc.tile_pool(name="attn", bufs=4))
        rcpp = ep(tc.tile_pool(name="rcp", bufs=2))
        outrow = ep(tc.tile_pool(name="orow", bufs=2))

        # PSUM pools: 2 + 2 + 1 + 2 + 1 = 8 banks, alive for the whole program
        pa = ep(tc.tile_pool(name="pa", bufs=2, space=PSUM))    # A q|k + C proj
        ps = ep(tc.tile_pool(name="ps", bufs=2, space=PSUM))    # B sc + A v + C proj
        pt = ep(tc.tile_pool(name="pt", bufs=1, space=PSUM))    # A transposes
        py = ep(tc.tile_pool(name="py", bufs=2, space=PSUM))    # B y accum
        pm = ep(tc.tile_pool(name="pm", bufs=1, space=PSUM))    # B softmax sums

        for b in range(B):
            qkT = qkt_pool.tile([P, 4 * T], F16, tag="qkT", name=f"qkT_{b}")
            v_sb = v_pool.tile([P, MT * HPC * D], F16, tag="v", name=f"v_{b}")
            yt = yt_pool.tile([P, HPC * T], F16, tag="yt", name=f"yt_{b}")
            qv = qkT[:].rearrange("p (hb t) -> p hb t", hb=4)

            # -------- emission helpers --------
            def emit_A_group(g):
                halves = []
                tsl = slice(b * T + g * GM * P, b * T + (g + 1) * GM * P)
                for hf in range(2):
                    xc = xcol.tile([P, KH, GM * P], F16, tag="xc",
                                   name=f"xc_{b}_{g}_{hf}")
                    if b == 0 and g == 0:
                        # contiguous per-kt loads: cheap triggers, so the
                        # first matmuls start ~8us in (the batched strided
                        # pattern pays a one-time slow descriptor path)
                        for k8 in range(KH):
                            nc.sync.dma_start(
                                out=xc[:, k8, :],
                                in_=xt_r[:, hf * KH + k8, tsl])
                    else:
                        nc.sync.dma_start(
                            out=xc[:], in_=xt_r[:, hf * KH:(hf + 1) * KH, tsl])
                    halves.append(xc)
                p5s = []
                pv = ps.tile([P, NQ], F32, tag="sc", name=f"pv_{b}_{g}")
                for mi in range(GM):
                    p5 = pa.tile([P, NQ], F32, tag="qk", name=f"p5_{b}_{g}_{mi}")
                    p5s.append(p5)
                    for kt in range(KTC):
                        lhsT = halves[kt // KH][:, kt % KH, mi * P:(mi + 1) * P]
                        nc.tensor.matmul(
                            p5[:], lhsT,
                            w_sb[:, kt * FPC: kt * FPC + 512],
                            start=(kt == 0), stop=(kt == KTC - 1))
                        nc.tensor.matmul(
                            pv[:, mi * 256:(mi + 1) * 256], lhsT,
                            w_sb[:, kt * FPC + 512:(kt + 1) * FPC],
                            start=(kt == 0 and mi == 0), stop=(kt == KTC - 1),
                            skip_group_check=True)
                # RoPE + transposes per m-tile of the group
                for mi in range(GM):
                    m = g * GM + mi
                    gm = b * MT + m
                    p5 = p5s[mi]
                    rot = rotp.tile([P, NQ], F16, tag="rot",
                                    name=f"rot_{b}_{g}_{mi}")
                    p3 = p5[:].rearrange("p (blk two d) -> p blk two d",
                                         two=2, d=RD)
                    re_, im_ = p3[:, :, 0, :], p3[:, :, 1, :]
                    r3 = rot[:].rearrange("p (blk two d) -> p blk two d",
                                          two=2, d=RD)
                    cosb = (cos_sb[:, gm * RD:(gm + 1) * RD]
                            .unsqueeze(1).broadcast_to([P, 4, RD]))
                    sinb = (sin_sb[:, gm * RD:(gm + 1) * RD]
                            .unsqueeze(1).broadcast_to([P, 4, RD]))
                    t1 = tmpp.tile([P, 256], F32, tag="t1")
                    t2 = tmpp.tile([P, 256], F32, tag="t2")
                    t1v = t1[:].rearrange("p (blk d) -> p blk d", d=RD)
                    t2v = t2[:].rearrange("p (blk d) -> p blk d", d=RD)
                    nc.vector.tensor_tensor(t1v, re_, cosb, MUL)
                    nc.vector.tensor_tensor(t2v, im_, sinb, MUL)
                    nc.vector.tensor_tensor(r3[:, :, 0, :], t1v, t2v, SUB)
                    t3 = tmpp.tile([P, 256], F32, tag="t3")
                    t4 = tmpp.tile([P, 256], F32, tag="t4")
                    t3v = t3[:].rearrange("p (blk d) -> p blk d", d=RD)
                    t4v = t4[:].rearrange("p (blk d) -> p blk d", d=RD)
                    nc.vector.tensor_tensor(t3v, re_, sinb, MUL)
                    nc.vector.tensor_tensor(t4v, im_, cosb, MUL)
                    nc.vector.tensor_tensor(r3[:, :, 1, :], t3v, t4v, ADD)
                    tp = pt.tile([P, 4 * P], F16, tag="tp",
                                 name=f"tp_{b}_{g}_{mi}")
                    for hb in range(4):
                        nc.tensor.matmul(
                            tp[:, hb * P:(hb + 1) * P],
                            rot[:, hb * P:(hb + 1) * P], ident[:],
                            is_transpose=True,
                            start=(hb == 0), stop=(hb == 3),
                            skip_group_check=True)
                    tpv = tp[:].rearrange("p (hb t) -> p hb t", hb=4)
                    eng = nc.scalar if state["nev"] % 2 == 0 else nc.vector
                    state["nev"] += 1
                    if eng is nc.scalar:
                        nc.scalar.copy(qv[:, :, m * P:(m + 1) * P], tpv)
                    else:
                        nc.vector.tensor_copy(qv[:, :, m * P:(m + 1) * P], tpv)
                eng = nc.scalar if state["nev"] % 2 == 0 else nc.vector
                state["nev"] += 1
                if eng is nc.scalar:
                    nc.scalar.copy(
                        v_sb[:, g * GM * HPC * D:(g + 1) * GM * HPC * D],
                        pv[:])
                else:
                    nc.vector.tensor_copy(
                        v_sb[:, g * GM * HPC * D:(g + 1) * GM * HPC * D],
                        pv[:])

            # -------- phase B/C with a cross-boundary software pipeline -----
            bq = deque()
            state = {"ncopy": 0, "nev": 0}

            def emit_C_chunk(j):
                for mi in range(4):
                    m = 4 * j + mi
                    orow = outrow.tile([P, C], F16, tag="orow",
                                       name=f"orow_{b}_{m}")
                    for oc in range(4):
                        pool = pa if oc % 2 == 0 else ps
                        tag = "qk" if oc % 2 == 0 else "sc"
                        op = pool.tile([P, NQ], F32, tag=tag,
                                       name=f"op_{b}_{m}_{oc}")
                        for h in range(HPC):
                            nc.tensor.matmul(
                                op[:],
                                yt[:, h * T + m * P: h * T + (m + 1) * P],
                                wp_sb[:, h * C + oc * 512:
                                      h * C + (oc + 1) * 512],
                                start=(h == 0), stop=(h == HPC - 1))
                        if state["ncopy"] % 2 == 0:
                            nc.scalar.copy(orow[:, oc * 512:(oc + 1) * 512],
                                           op[:])
                        else:
                            nc.vector.tensor_copy(
                                orow[:, oc * 512:(oc + 1) * 512], op[:])
                        state["ncopy"] += 1
                    nc.gpsimd.dma_start(
                        out=out_d[(b * MT + m) * P:(b * MT + m + 1) * P, :],
                        in_=orow[:])

            def pump():
                (h, j, kt, nkt, at, y_ps, s_ps, fr, qo) = bq.popleft()
                nc.tensor.matmul(
                    y_ps[:, qo:NQ],
                    v_sb[:, kt * HPC * D + h * D: kt * HPC * D + (h + 1) * D],
                    at[:, 0:fr], start=(kt == 0), stop=(kt == nkt - 1),
                    skip_group_check=True)
                nc.tensor.matmul(
                    s_ps[:, qo:NQ], ones_c[:], at[:, 0:fr],
                    start=(kt == 0), stop=(kt == nkt - 1),
                    skip_group_check=True)
                if kt == nkt - 1:
                    inst = b * HPC + h
                    rcp = rcpp.tile([P, NQ], F32, tag="rc",
                                    name=f"rc_{inst}_{j}")
                    with nc.allow_low_precision(reason="softmax recip"):
                        nc.vector.reciprocal_approx_fast(out=rcp[:],
                                                         in_=s_ps[:])
                    nc.vector.tensor_tensor(
                        yt[:, h * T + j * NQ: h * T + (j + 1) * NQ],
                        y_ps[:], rcp[:], MUL)
                    if h == HPC - 1:
                        emit_C_chunk(j)

            def emit_B_chunk(j):
                nkt = 4 * (j + 1)
                for h in range(HPC):
                    inst = b * HPC + h
                    y_ps = py.tile([P, NQ], F32, tag="y", name=f"y_{inst}_{j}")
                    s_ps = pm.tile([P, NQ], F32, tag="s", name=f"s_{inst}_{j}")
                    for kt in range(nkt):
                        kd = kt - 4 * j
                        fr = NQ if kd < 0 else NQ - kd * P
                        qo = 0 if kd < 0 else kd * P
                        sc = ps.tile([P, NQ], F32, tag="sc",
                                     name=f"sc_{inst}_{j}_{kt}")
                        nc.tensor.matmul(
                            sc[:, 0:fr],
                            qkT[:, (2 + h) * T + kt * P:
                                (2 + h) * T + (kt + 1) * P],
                            qkT[:, h * T + j * NQ + qo: h * T + (j + 1) * NQ],
                            start=True, stop=True)
                        at = attnp.tile([P, NQ], F16, tag="at",
                                        name=f"at_{inst}_{j}_{kt}")
                        nc.scalar.activation(at[:, 0:fr], sc[:, 0:fr], EXP,
                                             scale=SCALE)
                        if kd >= 0:
                            nc.gpsimd.tensor_tensor(
                                at[:, 0:P], at[:, 0:P], tri_sb[:], MUL)
                        bq.append((h, j, kt, nkt, at, y_ps, s_ps, fr, qo))
                        while len(bq) >= 2:
                            pump()

            # -------- batch emission schedule --------
            for g in range(NG):
                emit_A_group(g)
                if g % 2 == 1:
                    emit_B_chunk(g // 2)
            while bq:
                pump()

    nc.compile()
    return nc


def _perm(rows):
    return np.concatenate([rows[0::2], rows[1::2]], axis=0)


def _host_inputs(x, mask, freqs_cos, freqs_sin, w_attn, w_proj):
    f16 = np.float16
    f32 = np.float32
    x = np.asarray(x, f32)
    fc = np.asarray(freqs_cos, f32)
    fs = np.asarray(freqs_sin, f32)
    w_attn = np.asarray(w_attn, f32)
    w_proj = np.asarray(w_proj, f32)

    xT = np.ascontiguousarray(x.reshape(BT, C).T.astype(f16))

    def rows_arrange(a):  # [BT, RD] -> [P, (BT//P)*RD]
        return np.ascontiguousarray(
            a.reshape(BT // P, P, RD).transpose(1, 0, 2).reshape(P, -1)
            .astype(f16))

    cosw = rows_arrange(np.concatenate([fc] * B, axis=0))
    sinw = rows_arrange(np.concatenate([fs] * B, axis=0))

    # triangular keep-mask in [k, q] orientation: keep iff k' <= q'
    trid = np.triu(np.ones((P, P), dtype=f16))
    trid = np.ascontiguousarray(trid)

    wq, wk, wv = w_attn[0:C], w_attn[C:2 * C], w_attn[2 * C:3 * C]
    in_maps = []
    for c in range(NCORES):
        h0, h1 = HPC * c, HPC * c + 1
        Wc = np.concatenate([
            _perm(wq[h0 * D:(h0 + 1) * D]), _perm(wq[h1 * D:(h1 + 1) * D]),
            _perm(wk[h0 * D:(h0 + 1) * D]), _perm(wk[h1 * D:(h1 + 1) * D]),
            wv[h0 * D:(h0 + 1) * D], wv[h1 * D:(h1 + 1) * D]], axis=0)
        wqkv_c = np.ascontiguousarray(
            Wc.T.reshape(KTC, P, FPC).transpose(1, 0, 2).reshape(P, KTC * FPC)
            .astype(f16))
        wp_c = w_proj[:, c * HPC * D:(c + 1) * HPC * D].T  # [256, C]
        wp_c = np.ascontiguousarray(
            wp_c.reshape(HPC, P, C).transpose(1, 0, 2).reshape(P, HPC * C)
            .astype(f16))
        in_maps.append({
            "xt": xT, "wqkv": wqkv_c, "cosw": cosw, "sinw": sinw,
            "trid": trid, "wproj": wp_c,
        })
    return in_maps


def kernel(x, mask, freqs_cos, freqs_sin, w_attn, w_proj):
    global _PROGRAM
    _ensure_concourse()
    from concourse.bass_utils import run_bass_kernel_spmd

    if _PROGRAM is None:
        _PROGRAM = _build_program()
    nc = _PROGRAM

    in_maps = _host_inputs(x, mask, freqs_cos, freqs_sin, w_attn, w_proj)
    res = run_bass_kernel_spmd(nc, in_maps, list(range(NCORES)))
    out = res.results[0]["outp"].astype(np.float32)
    for i in range(1, NCORES):
        out = out + res.results[i]["outp"].astype(np.float32)
    return np.ascontiguousarray(out.reshape(B, T, C))
